# revision 23
# baseline (speedup 1.0000x reference)
"""Trainium2 Bass kernel for nn_CMB_H_OMBH2 (MLP -> natural cubic spline -> grid eval).

Strategy (v3):
  - Grid rows are mirror-symmetric (fftfreq^2): row i == row 256-i.  Only rows
    0..128 are unique.  Core c computes unique rows [16c, 16c+17); the host
    places each computed row at both mirror positions during gather/unshard.
  - Spline solve restructured as matmuls only:
      val[c, n] = sum_i y_t[i, c] * BB[i, n],   BB = F1^T u + F2^T s + F3^T p
    with u = clip(x - kn_j, 0, h_j), s = u^2, p = u^3 (truncated-power basis),
    F_k = G_k A^-1 R (127 x 128) built on device from the knots input via a
    symmetrized Neumann-product inverse (16 terms, ||E~|| <= 0.52).
  - y_t (knot-major) produced directly by the last MLP layer via a stride-2
    lhsT view of h2; b3 folded in with a ones row; a0 folded via a bias column
    on the BB PSUM->SBUF copy.
  - All wide matmuls in f32r (1 cycle/row at >=256 cols); weights and grid
    rows enter as f32r DRAM tensors so no engine conversion copies are needed.
  - Diagonal-band matrices built as (host 0/1 mask) * (knot-value column)
    tensor_scalar ops, spread across DVE/Pool.
"""
import sys
import numpy as np

sys.path.insert(0, "/opt/trn_rl_repo")

N_CORES = 8
ROWS_PER_CORE = 17          # unique grid rows per core (1 overlap)
CHUNK = 512
NPTS = 4352                 # 17*256: chunk 0 = 1 row, chunks 1..8 = 2 rows
N_CHUNKS = 9
THETA_LO = (50.0, 0.0075)
THETA_SCALE = (40.0, 0.0492)
BIG = 3.0e38

_CACHE = {}


def _chunk_geom(ci):
    """(point offset, n points, first output row) for chunk ci."""
    if ci == 0:
        return 0, 256, 0
    return 256 + (ci - 1) * CHUNK, CHUNK, 2 * ci - 1


def _build_program():
    import concourse.bacc as bacc
    import concourse.tile as tile
    import concourse.mybir as mybir

    dt = mybir.dt
    Alu = mybir.AluOpType
    Act = mybir.ActivationFunctionType

    nc = bacc.Bacc("TRN2", target_bir_lowering=False, debug=False,
                   num_devices=N_CORES)
    f32 = dt.float32
    f32r = dt.float32r

    kn4_d = nc.dram_tensor("kn4", [128, 4], f32, kind="ExternalInput").ap()
    pk1_d = nc.dram_tensor("pk1", [128, 776], f32, kind="ExternalInput").ap()
    pkw_d = nc.dram_tensor("pkw", [128, 432], f32r, kind="ExternalInput").ap()
    pk0_d = nc.dram_tensor("pk0", [2, 258], f32, kind="ExternalInput").ap()
    xrow_d = nc.dram_tensor("xrow", [1, NPTS], f32r, kind="ExternalInput").ap()
    ones_d = nc.dram_tensor("onesr", [1, NPTS], f32r, kind="ExternalInput").ap()
    out_d = nc.dram_tensor("out", [256, ROWS_PER_CORE, 256], f32,
                           kind="ExternalOutput").ap()

    with tile.TileContext(nc) as tc:
        with (
            tc.tile_pool(name="const", bufs=1) as cp,
            tc.tile_pool(name="ucpl", bufs=4) as ucp,
            tc.tile_pool(name="stpl", bufs=3) as stp,
            tc.tile_pool(name="ptpl", bufs=3) as ptp,
            tc.tile_pool(name="bbpl", bufs=3) as bbp,
            tc.tile_pool(name="obpl", bufs=4) as obp,
            tc.tile_pool(name="zps", bufs=2, space="PSUM") as zps,
            tc.tile_pool(name="bps", bufs=2, space="PSUM") as bps,
            tc.tile_pool(name="vps", bufs=2, space="PSUM") as vps,
        ):
            # ---------------- input DMAs (sync queue, priority order) ----
            kn4 = cp.tile([128, 4], f32)
            nc.sync.dma_start(kn4[:], kn4_d[:])
            pk1 = cp.tile([128, 776], f32)
            nc.sync.dma_start(pk1[:], pk1_d[:])
            pkw = cp.tile([128, 432], f32r)
            nc.sync.dma_start(pkw[:], pkw_d[:])
            pk0 = cp.tile([2, 258], f32)
            nc.sync.dma_start(pk0[:], pk0_d[:])
            xr2 = cp.tile([2, NPTS], f32r)
            nc.sync.dma_start(xr2[0:1, :], xrow_d[:])
            nc.sync.dma_start(xr2[1:2, :], ones_d[:])

            sd_s = pk1[:, 0:128]
            id_s = pk1[:, 128:256]
            mm1 = pk1[:, 256:384]       # mask j = q-1
            m0 = pk1[:, 384:512]        # mask j = q
            m1 = pk1[:, 512:640]        # mask j = q+1
            m2 = pk1[:, 640:768]        # mask j = q+2
            b0c = pk1[0:100, 768:769]
            b1c = pk1[0:100, 769:770]
            b2c = pk1[0:100, 770:771]
            bigz = pk1[:, 771:772]
            e0col = pk1[:, 772:773]
            w1_s = pkw[0:100, 0:100]
            w2_s = pkw[0:100, 100:200]
            w3_s = pkw[0:101, 200:328]
            w0_s = pkw[0:2, 328:428]

            # ---------------- per-knot columns (DVE chain) ---------------
            k0 = kn4[:, 0:1]
            k1 = kn4[:, 1:2]
            k2 = kn4[:, 2:3]
            k3 = kn4[:, 3:4]
            cols = cp.tile([128, 24], f32)
            h_c = cols[:, 0:1]
            h1_c = cols[:, 1:2]
            h2_c = cols[:, 2:3]
            t2 = cols[:, 3:5]
            sq2 = cols[:, 5:7]
            rq2 = cols[:, 7:9]
            rh_c = cols[:, 9:10]
            rh1_c = cols[:, 10:11]
            etmp = cols[:, 11:12]
            e_c = cols[:, 12:13]
            caps = cols[:, 13:14]
            nk0 = cols[:, 14:15]
            ra_c = cols[:, 15:16]
            rbt = cols[:, 16:17]
            rb_c = cols[:, 17:18]
            rc_c = cols[:, 18:19]
            ga_c = cols[:, 19:20]
            gb_c = cols[:, 20:21]
            gc_c = cols[:, 21:22]
            ca_c = cols[:, 22:23]
            cb_c = cols[:, 23:24]
            wyn = cp.tile([128, 1], f32)
            eS_c = cp.tile([128, 1], f32)
            rhS_c = cp.tile([128, 1], f32)
            rsq = rq2[:, 0:1]
            rsq1 = rq2[:, 1:2]

            nc.vector.tensor_tensor(h_c, k1, k0, Alu.subtract)
            nc.vector.tensor_tensor(h1_c, k2, k1, Alu.subtract)
            nc.vector.tensor_tensor(h2_c, k3, k2, Alu.subtract)
            nc.vector.tensor_tensor(t2[:, 0:1], h_c, h1_c, Alu.add)
            nc.vector.tensor_tensor(t2[:, 1:2], h1_c, h2_c, Alu.add)
            # clamp keeps junk tail rows (knot padding) positive: sqrt(neg)=nan
            # would poison the shift matmul (0*nan=nan).  Valid rows are >= 8.
            nc.vector.tensor_scalar(t2[:], t2[:], 1.0, None, Alu.max)
            nc.scalar.activation(sq2[:], t2[:], Act.Sqrt, scale=2.0)
            nc.vector.reciprocal(rq2[:], sq2[:])
            nc.vector.reciprocal(rh_c, h_c)
            nc.vector.reciprocal(rh1_c, h1_c)
            nc.vector.tensor_tensor(etmp, h1_c, rsq, Alu.mult)
            nc.vector.scalar_tensor_tensor(e_c, etmp, -1.0, rsq1, Alu.mult,
                                           Alu.mult)
            nc.vector.scalar_tensor_tensor(ra_c, rh_c, 6.0, rsq, Alu.mult,
                                           Alu.mult)
            nc.vector.tensor_tensor(rbt, rh_c, rh1_c, Alu.add)
            nc.vector.scalar_tensor_tensor(rb_c, rbt, -6.0, rsq, Alu.mult,
                                           Alu.mult)
            nc.vector.scalar_tensor_tensor(rc_c, rh1_c, 6.0, rsq, Alu.mult,
                                           Alu.mult)

            # eS = Sd^T @ e (shift down one partition)
            eps_ps = zps.tile([128, 1], f32, tag="zp")
            nc.tensor.matmul(eps_ps[:], sd_s, e_c, start=True, stop=True)
            nc.scalar.copy(eS_c[:], eps_ps[:])

            # ---------------- E~ / R~ into U-chain rhs0 ------------------
            rhs = [cp.tile([126, 256], f32r, name=f"rhs{i}") for i in range(4)]
            zpad = cp.tile([126, 2], f32)
            nc.gpsimd.memset(zpad[:], 0.0)
            for t_ in rhs:
                nc.gpsimd.tensor_copy(t_[:, 254:256], zpad[:])
            esc = cp.tile([126, 126], f32)
            nc.vector.tensor_scalar(esc[:], mm1[0:126, 0:126], eS_c[0:126, :],
                                    None, Alu.mult)
            nc.vector.scalar_tensor_tensor(rhs[0][:, 0:126], m1[0:126, 0:126],
                                           e_c[0:126, :], esc[:],
                                           Alu.mult, Alu.add)
            rsc = cp.tile([126, 128], f32)
            nc.vector.tensor_scalar(rsc[:], m0[0:126, :], ra_c[0:126, :],
                                    None, Alu.mult)
            nc.vector.scalar_tensor_tensor(rsc[:], m1[0:126, :],
                                           rb_c[0:126, :], rsc[:],
                                           Alu.mult, Alu.add)
            nc.vector.scalar_tensor_tensor(rhs[0][:, 126:254], m2[0:126, :],
                                           rc_c[0:126, :], rsc[:],
                                           Alu.mult, Alu.add)

            # deferred per-knot columns (needed only after the U-chain)
            nc.vector.tensor_tensor(caps, h_c, bigz, Alu.add)
            nc.vector.tensor_scalar_mul(nk0, k0, -1.0)
            nc.vector.scalar_tensor_tensor(ga_c, rh_c, 1.0 / 6.0, rsq,
                                           Alu.mult, Alu.mult)
            nc.vector.scalar_tensor_tensor(gb_c, rh1_c, -1.0 / 6.0, rsq,
                                           Alu.mult, Alu.mult)
            nc.vector.tensor_scalar_mul(gc_c, rsq, 0.5)
            nc.vector.scalar_tensor_tensor(ca_c, h_c, -1.0 / 6.0, rsq,
                                           Alu.mult, Alu.mult)
            nc.vector.scalar_tensor_tensor(cb_c, h1_c, -1.0 / 3.0, rsq,
                                           Alu.mult, Alu.mult)
            nc.vector.tensor_scalar_mul(wyn[:], rh_c, -1.0)
            rhs_ps = zps.tile([128, 1], f32, tag="zp")
            nc.tensor.matmul(rhs_ps[:], sd_s, rh_c, start=True, stop=True)
            nc.scalar.copy(rhS_c[:], rhs_ps[:])
            knm = cp.tile([128, 2], f32)
            nc.vector.memset(knm[:, 0:1], 1.0)      # multiplies the x row
            nc.vector.tensor_copy(knm[:, 1:2], nk0)  # multiplies the ones row
            knw_ps = zps.tile([2, 128], f32, tag="zp")
            nc.tensor.transpose(knw_ps[:], knm[:], id_s)
            knw = cp.tile([2, 128], f32r)
            nc.scalar.copy(knw[:], knw_ps[:])

            # G-transpose band matrices (Pool, mask * broadcast column)
            g3t = cp.tile([126, 127], f32)
            g3b = cp.tile([126, 127], f32)
            nc.gpsimd.tensor_tensor(g3t[:], m0[0:126, 0:127],
                                    ga_c[0:126, :].broadcast_to([126, 127]),
                                    Alu.mult)
            nc.gpsimd.tensor_tensor(g3b[:], m1[0:126, 0:127],
                                    gb_c[0:126, :].broadcast_to([126, 127]),
                                    Alu.mult)
            nc.gpsimd.tensor_tensor(g3t[:], g3t[:], g3b[:], Alu.add)
            g2t = cp.tile([126, 127], f32)
            nc.gpsimd.tensor_tensor(g2t[:], m1[0:126, 0:127],
                                    gc_c[0:126, :].broadcast_to([126, 127]),
                                    Alu.mult)
            cct = cp.tile([126, 127], f32)
            ccb = cp.tile([126, 127], f32)
            nc.gpsimd.tensor_tensor(cct[:], m0[0:126, 0:127],
                                    ca_c[0:126, :].broadcast_to([126, 127]),
                                    Alu.mult)
            nc.gpsimd.tensor_tensor(ccb[:], m1[0:126, 0:127],
                                    cb_c[0:126, :].broadcast_to([126, 127]),
                                    Alu.mult)
            nc.gpsimd.tensor_tensor(cct[:], cct[:], ccb[:], Alu.add)
            # W1y^T: [i, i] = -rh_i ; [i, i-1] = rh_{i-1} (shifted col)
            w1yt = cp.tile([128, 127], f32)
            w1ytb = cp.tile([128, 127], f32)
            nc.gpsimd.tensor_tensor(w1yt[:], m0[:, 0:127],
                                    wyn[:].broadcast_to([128, 127]),
                                    Alu.mult)
            nc.gpsimd.tensor_tensor(w1ytb[:], mm1[:, 0:127],
                                    rhS_c[:].broadcast_to([128, 127]),
                                    Alu.mult)
            nc.gpsimd.tensor_tensor(w1yt[:], w1yt[:], w1ytb[:], Alu.add)

            # ---------------- MLP ---------------------------------------
            thetaT = pk0[:, 0:256]
            lo_c = pk0[:, 256:257]
            isc_c = pk0[:, 257:258]
            tn = cp.tile([2, 256], f32r)
            nc.vector.tensor_scalar(tn[:], thetaT, lo_c, isc_c,
                                    Alu.subtract, Alu.mult)
            h0 = cp.tile([100, 256], f32r)
            h1t = cp.tile([100, 256], f32r)
            h2e = cp.tile([101, 256], f32r)
            nc.sync.dma_start(h2e[100:101, :], ones_d[:, 0:256])  # b3 fold row
            l0ps = bps.tile([100, 256], f32, tag="bb")
            nc.tensor.matmul(l0ps[:], w0_s, tn[:], start=True, stop=True)
            nc.scalar.activation(h0[:], l0ps[:], Act.Relu, bias=b0c)
            l1ps = bps.tile([100, 256], f32, tag="bb")
            nc.tensor.matmul(l1ps[:], w1_s, h0[:], start=True, stop=True)
            nc.scalar.activation(h1t[:], l1ps[:], Act.Relu, bias=b1c)
            l2ps = vps.tile([100, 256], f32, tag="vp")
            nc.tensor.matmul(l2ps[:], w2_s, h1t[:], start=True, stop=True)
            nc.scalar.activation(h2e[0:100, :], l2ps[:], Act.Relu, bias=b2c)
            h2v = h2e[:].rearrange("p (i t) -> p t i", t=2)
            y0ps = vps.tile([128, 128], f32, tag="vp")
            nc.tensor.matmul(y0ps[:], h2v[:, 0, :], w3_s, start=True,
                             stop=True)
            y1ps = zps.tile([128, 128], f32, tag="zp")
            nc.tensor.matmul(y1ps[:], h2v[:, 1, :], w3_s, start=True,
                             stop=True)
            y_t = cp.tile([128, 256], f32r)
            nc.scalar.copy(y_t[:, 0:128], y0ps[:])
            nc.vector.tensor_copy(y_t[:, 128:256], y1ps[:])

            # ---------------- U-chain (4 stages) -------------------------
            for st in range(4):
                ups = bps.tile([126, 256], f32, tag="bb")
                nc.tensor.matmul(ups[:], rhs[st][:, 0:126], rhs[st][:],
                                 start=True, stop=True)
                if st < 3:
                    nc.scalar.copy(rhs[st + 1][:, 0:126], ups[:, 0:126])
                    nc.vector.tensor_tensor(rhs[st + 1][:, 126:254],
                                            rhs[st][:, 126:254],
                                            ups[:, 126:254], Alu.add)
                else:
                    u4 = cp.tile([126, 128], f32)
                    nc.vector.tensor_tensor(u4[:], rhs[st][:, 126:254],
                                            ups[:, 126:254], Alu.add)

            # ---------------- F^T matrices and W weights ----------------
            # Fk^T = U4^T @ Gk^T  (U4 = P4 R~, P sym)
            f3ps = vps.tile([128, 127], f32, tag="vp")
            nc.tensor.matmul(f3ps[:], u4[:], g3t[:], start=True, stop=True)
            f2ps = zps.tile([128, 127], f32, tag="zp")
            nc.tensor.matmul(f2ps[:], u4[:], g2t[:], start=True, stop=True)
            fcps = bps.tile([128, 127], f32, tag="bb")
            nc.tensor.matmul(fcps[:], u4[:], cct[:], start=True, stop=True)
            f3t = cp.tile([128, 127], f32r)
            nc.scalar.copy(f3t[:], f3ps[:])
            f2t = cp.tile([128, 127], f32r)
            nc.vector.tensor_copy(f2t[:], f2ps[:])
            f1t = cp.tile([128, 128], f32r)
            nc.vector.tensor_tensor(f1t[:, 0:127], w1yt[:], fcps[:], Alu.add)
            nc.vector.tensor_copy(f1t[:, 127:128], e0col)  # a0 row selector

            # W weights: Wk = Fk @ y_t  (plus a0 row in W1)
            w1ps = vps.tile([128, 256], f32, tag="vp")
            nc.tensor.matmul(w1ps[:], f1t[:], y_t[:], start=True, stop=True)
            w2ps = zps.tile([127, 256], f32, tag="zp")
            nc.tensor.matmul(w2ps[:], f2t[:], y_t[:], start=True, stop=True)
            w3ps = bps.tile([127, 256], f32, tag="bb")
            nc.tensor.matmul(w3ps[:], f3t[:], y_t[:], start=True, stop=True)
            w1w = cp.tile([128, 256], f32r)
            nc.scalar.copy(w1w[:], w1ps[:])
            w2w = cp.tile([127, 256], f32r)
            nc.vector.tensor_copy(w2w[:], w2ps[:])
            w3w = cp.tile([127, 256], f32r)
            nc.scalar.copy(w3w[:], w3ps[:])

            # ---------------- eval loop (software-pipelined) -------------
            out_v = out_d.rearrange("(a p) r c -> p a r c", a=2)
            uc_bufs = [cp.tile([128, CHUNK], f32r, name=f"ucb{k}")
                       for k in range(4)]
            for k in range(4):
                nc.sync.dma_start(uc_bufs[k][127:128, :], ones_d[:, 0:CHUNK])

            def emit_z_uc(ci):
                s0, npt, _ = _chunk_geom(ci)
                zp = zps.tile([128, CHUNK], f32, tag="zp", name=f"zp{ci}")
                nc.tensor.matmul(zp[:, 0:npt], knw[:], xr2[:, s0:s0 + npt],
                                 start=True, stop=True)
                uc = uc_bufs[ci % 4]
                nc.vector.tensor_scalar(uc[0:127, 0:npt], zp[0:127, 0:npt],
                                        0.0, caps[0:127, :], Alu.max, Alu.min)
                return uc

            ucs = {0: emit_z_uc(0)}
            for ci in range(N_CHUNKS):
                _, npt, r0 = _chunk_geom(ci)
                uc = ucs.pop(ci)
                s_t = stp.tile([127, CHUNK], f32r, tag="st", name=f"st{ci}")
                nc.gpsimd.tensor_tensor(s_t[:, 0:npt], uc[0:127, 0:npt],
                                        uc[0:127, 0:npt], Alu.mult)
                if ci + 1 < N_CHUNKS:
                    ucs[ci + 1] = emit_z_uc(ci + 1)
                p_t = ptp.tile([127, CHUNK], f32r, tag="pt", name=f"pt{ci}")
                nc.vector.tensor_tensor(p_t[:, 0:npt], uc[0:127, 0:npt],
                                        s_t[:, 0:npt], Alu.mult)
                vv = vps.tile([128, 2 * CHUNK], f32, tag="vp")
                for half in range(2):
                    cs = slice(128 * half, 128 * half + 128)
                    vs = slice(CHUNK * half, CHUNK * half + npt)
                    nc.tensor.matmul(vv[:, vs], w1w[:, cs], uc[:, 0:npt],
                                     start=True, stop=False)
                    nc.tensor.matmul(vv[:, vs], w2w[:, cs], s_t[:, 0:npt],
                                     start=False, stop=False)
                    nc.tensor.matmul(vv[:, vs], w3w[:, cs], p_t[:, 0:npt],
                                     start=False, stop=True)
                ob = obp.tile([128, 2 * CHUNK], f32, tag="ob")
                nc.scalar.copy(ob[:, 0:CHUNK], vv[:, 0:CHUNK])
                nc.scalar.copy(ob[:, CHUNK:2 * CHUNK], vv[:, CHUNK:2 * CHUNK])
                obv = ob[:].rearrange("p (a r c) -> p a r c", a=2, r=2)
                if ci == 0:
                    nc.sync.dma_start(out_v[:, :, 0:1, :], obv[:, :, 0:1, :])
                else:
                    nc.sync.dma_start(out_v[:, :, r0:r0 + 2, :], obv[:])
    nc.compile()
    return nc


def _round_f32r(a):
    # f32r keeps fp32 bits; PE reads them at reduced internal precision.
    # No host rounding needed -- dtype tag only.
    return np.ascontiguousarray(a, np.float32)


def _host_pack(inputs):
    f = np.float32
    theta = np.asarray(inputs["theta"], f)
    W0 = np.asarray(inputs["W0"], f)
    b0 = np.asarray(inputs["b0"], f)
    W1 = np.asarray(inputs["W1"], f)
    b1 = np.asarray(inputs["b1"], f)
    W2 = np.asarray(inputs["W2"], f)
    b2 = np.asarray(inputs["b2"], f)
    W3 = np.asarray(inputs["W3"], f)
    b3 = np.asarray(inputs["b3"], f)
    knots = np.asarray(inputs["knots"], f)

    kn4 = np.zeros((128, 4), f)
    for s in range(4):
        kn4[:128 - s, s] = knots[s:]

    pk1 = np.zeros((128, 776), f)
    sd = np.zeros((128, 128), f)
    for q in range(1, 128):
        sd[q - 1, q] = 1.0
    pk1[:, 0:128] = sd
    pk1[:, 128:256] = np.eye(128, dtype=f)
    for q in range(128):                      # band masks
        if q - 1 >= 0:
            pk1[q, 256 + q - 1] = 1.0         # Mm1: j = q-1
        pk1[q, 384 + q] = 1.0                 # M0: j = q
        if q + 1 < 128:
            pk1[q, 512 + q + 1] = 1.0         # M1: j = q+1
        if q + 2 < 128:
            pk1[q, 640 + q + 2] = 1.0         # M2: j = q+2
    pk1[0:100, 768] = b0
    pk1[0:100, 769] = b1
    pk1[0:100, 770] = b2
    pk1[126, 771] = BIG
    pk1[0, 772] = 1.0

    pkw = np.zeros((128, 432), f)
    pkw[0:100, 0:100] = W1
    pkw[0:100, 100:200] = W2
    pkw[0:100, 200:328] = W3
    pkw[100, 200:328] = b3
    pkw[0:2, 328:428] = W0

    pk0 = np.zeros((2, 258), f)
    pk0[:, 0:256] = theta.T
    pk0[0, 256] = THETA_LO[0]
    pk0[1, 256] = THETA_LO[1]
    pk0[0, 257] = 1.0 / np.float32(THETA_SCALE[0])
    pk0[1, 257] = 1.0 / np.float32(THETA_SCALE[1])

    onesr = np.ones((1, NPTS), f)
    return kn4, pk1, _round_f32r(pkw), pk0, _round_f32r(onesr)


def kernel(**inputs):
    from concourse.bass_utils import run_bass_kernel_spmd

    if "nc" not in _CACHE:
        _CACHE["nc"] = _build_program()
    nc = _CACHE["nc"]

    grid = np.ascontiguousarray(np.asarray(inputs["grid"], np.float32))
    kn4, pk1, pkw, pk0, onesr = _host_pack(inputs)
    common = dict(kn4=kn4, pk1=pk1, pkw=pkw, pk0=pk0, onesr=onesr)

    in_maps = []
    for c in range(N_CORES):
        rows = grid[16 * c:16 * c + ROWS_PER_CORE]
        m = dict(common)
        m["xrow"] = _round_f32r(rows.reshape(1, -1))
        in_maps.append(m)

    res = run_bass_kernel_spmd(nc, in_maps, list(range(N_CORES)),
                               trace=bool(_CACHE.get("trace", False)),
                               tmpdir=_CACHE.get("tmpdir"))
    _CACHE["last_res"] = res

    full = np.empty((256, 256, 256), np.float32)
    for r in range(129):
        c = min(r // 16, 7)
        full[:, r, :] = res.results[c]["out"][:, r - 16 * c, :]
    for r in range(129, 256):
        full[:, r, :] = full[:, 256 - r, :]
    return full
